# revision 70
# baseline (speedup 1.0000x reference)
"""Trainium2 Bass kernel for AdaptiveGraphAttentionModule.

Model (per reference):
  q,k,v = linear projections of x, split into 8 heads of dim 16
  scores = q@k^T * (1/4) + adj          (adj broadcast over batch)
  attn   = softmax(scores, axis=-1)
  out    = (attn @ v) -> merge heads -> linear (Wo, bo)
  loss   = 1e-4 * mean(|attn|)          (== 1e-4/N exactly: softmax rows sum to 1)

Distribution: data-parallel over batch. 8 cores x 2 batches each; weights and
adj replicated. No collectives.

Device-side design (per core; "T" = feature-on-partitions transposed):
  - scores are computed transposed (scoresT[j,i] = sum_d k[j,d] q[i,d]) so the
    resulting attn (key index j on partitions) streams directly into the
    attn@v matmul with no transpose of the big N*N tensor.
  - heads are processed in pairs on adjacent 32-partition bands so their K=16
    matmuls can overlap in the PE array (32-row tile_position groups).
  - per (head, batch), jt key-tiles 0..NJA-1 take the exact route: exp(s) on
    ScalarE, then a bf16 VectorE multiply by E = exp(adj) (precomputed once
    per head on ScalarE while the adj tile is staged, exploiting
    exp(s+a) = exp(s)*exp(a)). Tiles NJA..7 take the cheap route: one VectorE
    op computes
    round(scores*128*log2(e) + (adj*128*log2(e) + 16250.5)) as int16, whose
    bits reinterpreted as bf16 are 2^((s+adj)*log2e) = exp(s+adj) to ~3%
    (Schraudolph), folding adj-add + exp + bf16-cast into a single pass.
  - the softmax denominator comes free from an extra ones-column in V (the
    same matmuls that compute attn@v also accumulate the row sums); bv is
    folded into V via a rank-1 accumulating matmul so the normalized output
    picks it up exactly. 1/denominator runs on ScalarE as exp(-ln(|x|+eps))
    and is broadcast to each band by one k=128 selector matmul.
  - attn@v and the normalization consumers are software-pipelined one step
    behind their producers so the PE's strict FIFO never waits on the
    ScalarE/VectorE round trips.
"""

import os
import sys

import numpy as np

for _p in ("/opt/trn_rl_repo",):
    if os.path.isdir(_p) and _p not in sys.path:
        sys.path.insert(0, _p)

import concourse.bacc as bacc
import concourse.mybir as mybir
from concourse import bass_utils, tile

AF = mybir.ActivationFunctionType
ALU = mybir.AluOpType
DT = mybir.dt

N_CORES = 8
B, N, H = 16, 1024, 128
HEADS, HD = 8, 16
BL = B // N_CORES          # batches per core
NI = BL * N                # 2048 tokens per core
SCALE = 1.0 / 4.0          # 1/sqrt(HD)

LAST_EXEC_NS = None
_CACHED_NC = None
_HOOK_DONE = False


def _install_ntff_hook():
    """The agent image lacks ``antenv.axon_hooks``; recreate it so
    run_bass_kernel_spmd(trace=True) can reach the NTFF profiler."""
    global _HOOK_DONE
    if _HOOK_DONE:
        return
    _HOOK_DONE = True
    import types

    try:
        from antenv.axon_hooks import get_axon_ntff_profile_hook  # noqa: F401
    except ImportError:
        import antenv

        mod = types.ModuleType("antenv.axon_hooks")
        _h = {"hook": None}
        mod.set_axon_ntff_profile_hook = lambda h: _h.__setitem__("hook", h)
        mod.get_axon_ntff_profile_hook = lambda: _h["hook"]
        sys.modules["antenv.axon_hooks"] = mod
        antenv.axon_hooks = mod
    from antenv.axon_hooks import (
        get_axon_ntff_profile_hook,
        set_axon_ntff_profile_hook,
    )

    if get_axon_ntff_profile_hook() is None:
        try:
            from trn_agent_boot.trn_boot import _ntff_profile_via_ctypes

            set_axon_ntff_profile_hook(_ntff_profile_via_ctypes("/opt/axon/libaxon_pjrt.so"))
        except Exception:
            pass
    # artifact upload needs bucket credentials the container lacks
    bass_utils.upload_artifacts = lambda tmpdir: tmpdir


def _emit(nc, tc, stack):
    d = nc._io_aps

    cpool = stack.enter_context(tc.tile_pool(name="cpool", bufs=1))
    adjpool = stack.enter_context(tc.tile_pool(name="adjpool", bufs=2))
    adjbfpool = stack.enter_context(tc.tile_pool(name="adjbfpool", bufs=2))
    adjppool = stack.enter_context(tc.tile_pool(name="adjppool", bufs=2))
    attnpool = stack.enter_context(tc.tile_pool(name="attnpool", bufs=8))
    smallpool = stack.enter_context(tc.tile_pool(name="smallpool", bufs=3))
    pspool = stack.enter_context(tc.tile_pool(name="pspool", bufs=2, space="PSUM"))
    ppool2 = stack.enter_context(tc.tile_pool(name="ppool2", bufs=4, space="PSUM"))

    # ---- persistent inputs -------------------------------------------------
    xT = cpool.tile([128, NI], DT.float32)
    nc.sync.dma_start(out=xT[:, :], in_=d["xT"][:, :])
    wqk = cpool.tile([128, 256], DT.float32)
    nc.sync.dma_start(out=wqk[:, :], in_=d["wqk"][:, :])
    wvT = cpool.tile([128, 128], DT.float32)
    nc.sync.dma_start(out=wvT[:, :], in_=d["wvT"][:, :])
    woT = cpool.tile([128, 128], DT.float32)
    nc.sync.dma_start(out=woT[:, :], in_=d["woT"][:, :])
    bq_t = cpool.tile([128, 1], DT.float32)
    nc.sync.dma_start(out=bq_t[:, :], in_=d["bq2"][:, :])
    bk_t = cpool.tile([128, 1], DT.float32)
    nc.sync.dma_start(out=bk_t[:, :], in_=d["bk2"][:, :])
    bo_t = cpool.tile([128, 1], DT.float32)
    nc.sync.dma_start(out=bo_t[:, :], in_=d["bo2"][:, :])
    bv_t = cpool.tile([1, 128], DT.float32)
    nc.sync.dma_start(out=bv_t[:, :], in_=d["bvr"][:, :])

    woT_bf = cpool.tile([128, 128], DT.bfloat16)
    nc.vector.tensor_copy(woT_bf[:, :], woT[:, :])
    bqs = cpool.tile([128, 1], DT.float32)
    nc.vector.tensor_scalar_mul(bqs[:, :], bq_t[:, :], SCALE)

    # bf16 copies of everything that streams through the PE (fp32 matmuls run
    # LOW_HIGH dual-pass ~4x slower)
    xT_bf = cpool.tile([128, NI], DT.bfloat16)
    nc.vector.tensor_copy(xT_bf[:, :], xT[:, :])
    wqk_bf = cpool.tile([128, 256], DT.bfloat16)
    nc.vector.tensor_copy(wqk_bf[:, :], wqk[:, :])
    wvT_bf = cpool.tile([128, 128], DT.bfloat16)
    nc.vector.tensor_copy(wvT_bf[:, :], wvT[:, :])
    bv_bf = cpool.tile([1, 128], DT.bfloat16)
    nc.vector.tensor_copy(bv_bf[:, :], bv_t[:, :])

    ones_row = cpool.tile([1, 128], DT.bfloat16)
    nc.vector.memset(ones_row[:, :], 1.0)
    eps_c = cpool.tile([128, 1], DT.float32)
    nc.vector.memset(eps_c[:, :], 1e-30)
    zrow = cpool.tile([1, 512], DT.bfloat16)
    nc.vector.memset(zrow[:, :], 0.0)
    # denominator broadcast selector: e_sel2[k, m] = 1 iff k == 32*(m//32)+16,
    # i.e. one k=128 matmul copies each band's denominator row to all 32 rows
    # of that band (junk rec rows are multiplied by 0.0, and rec is built
    # NaN-free below so 0*junk == 0).
    esel_f = cpool.tile([128, 128], DT.float32)
    nc.sync.dma_start(out=esel_f[:, :], in_=d["esel"][:, :])
    e_sel = cpool.tile([128, 128], DT.bfloat16)
    nc.vector.tensor_copy(e_sel[:, :], esel_f[:, :])

    # ---- q/k projections ---------------------------------------------------
    # q_lin/k_lin: (feature h*16+d on partitions, token on free), bf16.
    # q is pre-scaled by 1/4.
    q_lin = cpool.tile([128, NI], DT.bfloat16)
    k_lin = cpool.tile([128, NI], DT.bfloat16)
    for t in range(NI // 512):
        sl = slice(t * 512, (t + 1) * 512)
        pq = ppool2.tile([128, 512], DT.float32, name=f"pq{t}", tag="p2")
        nc.tensor.matmul(pq[:, :], wqk_bf[:, 0:128], xT_bf[:, sl], start=True, stop=True)
        nc.scalar.activation(q_lin[:, sl], pq[:, :], AF.Identity, bias=bqs[:, :], scale=SCALE)
        pk = ppool2.tile([128, 512], DT.float32, name=f"pk{t}", tag="p2")
        nc.tensor.matmul(pk[:, :], wqk_bf[:, 128:256], xT_bf[:, sl], start=True, stop=True)
        nc.scalar.activation(k_lin[:, sl], pk[:, :], AF.Identity, bias=bk_t[:, :])

    # Rearrange q/k so head h sits at a 32-aligned partition base:
    # band g = h%4 holds heads {g, g+4} (disambiguated along free axis).
    q_band = cpool.tile([128, 2, NI], DT.bfloat16)
    k_band = cpool.tile([128, 2, NI], DT.bfloat16)
    for h in range(HEADS):
        g, hg = h % 4, h // 4
        nc.sync.dma_start(out=q_band[32 * g:32 * g + 16, hg, :], in_=q_lin[h * 16:(h + 1) * 16, :])
        nc.sync.dma_start(out=k_band[32 * g:32 * g + 16, hg, :], in_=k_lin[h * 16:(h + 1) * 16, :])

    # ---- v (natural layout: key index j on partitions) + ones column -------
    # v_aug[p, bl, h, jt, 0:16] = v rows, [..., 16] = 1.0 (softmax denominator)
    v_aug = cpool.tile([128, BL, HEADS, 8, 17], DT.bfloat16)
    nc.vector.memset(v_aug[:, :, :, :, :], 1.0)
    for bl in range(BL):
        for jt in range(8):
            pv = ppool2.tile([128, 512], DT.float32, name=f"pv{bl}{jt}", tag="p2")
            c0 = bl * N + jt * 128
            nc.tensor.matmul(pv[:, 0:128], xT_bf[:, c0:c0 + 128], wvT_bf[:, :], start=True, stop=False)
            nc.tensor.matmul(pv[:, 0:128], ones_row[:, :], bv_bf[:, :], start=False, stop=True)
            nc.vector.tensor_copy(
                v_aug[:, bl, :, jt, 0:16],
                pv[:, 0:128].rearrange("p (h d) -> p h d", h=HEADS),
            )

    # ---- attention main loop ------------------------------------------------

    # Main loop over head pairs: heads (2p, 2p+1) sit on adjacent 32-partition
    # bands so their K=16 scores matmuls can overlap in the PE array and their
    # attn@v matmuls go to adjacent PSUM col-bands of one shared tile.
    # Per (head, batch), jt tiles 0..NJA-1 take the exact route (adj added via
    # identity matmul, exp on ScalarE); tiles NJA..7 take the cheap route
    # (adj + exp2 bit-trick fused into one VectorE op producing bf16 bits).
    NJA = 3
    C1 = 128.0 * 1.4426950408889634          # 128*log2(e)
    BCONST = 16256.0 - 5.513                 # bf16 exponent bias - minimax shift
    avT = cpool.tile([128, NI], DT.bfloat16)
    deferred_norms = []

    def flush_norm(depth=2):
        # deferred two iterations so the bcast matmul never waits in the PE
        # FIFO for the ACT reciprocal chain (ACT's own queue delays it)
        if len(deferred_norms) <= depth:
            return
        fh0, fbl, frecs, fstashes = deferred_norms.pop(0)
        for ih in range(2):
            pbc = ppool2.tile([128, 512], DT.float32, name=f"pbc{fh0}{fbl}{ih}", tag="p2")
            nc.tensor.matmul(pbc[:, :], e_sel[:, :], frecs[ih][:, :], start=True, stop=True)
            bcsb = smallpool.tile([128, 512], DT.bfloat16, name="bcsb", tag="bcsb")
            nc.vector.tensor_copy(bcsb[:, :], pbc[:, :])
            tmp = smallpool.tile([128, 512], DT.bfloat16, name="tmp", tag="tmp")
            q_sl = slice(fbl * N + ih * 512, fbl * N + (ih + 1) * 512)
            for h in (fh0, fh0 + 1):
                g = h % 4
                nc.vector.scalar_tensor_tensor(tmp[32 * g:32 * g + 16, :],
                                               fstashes[ih][32 * g:32 * g + 16, :], 1.0,
                                               bcsb[32 * g:32 * g + 16, :],
                                               ALU.bypass, ALU.mult)
                nc.sync.dma_start(out=avT[h * 16:(h + 1) * 16, q_sl],
                                  in_=tmp[32 * g:32 * g + 16, :])

    for hp in range(HEADS // 2):
        h0 = 2 * hp
        adjbs, adjps = [], []
        for h in (h0, h0 + 1):
            adjb = adjbfpool.tile([128, NJA, 1024], DT.bfloat16, name=f"adjb{h}", tag="adjb")
            adjp = adjppool.tile([128, 8 - NJA, 1024], DT.int16, name=f"adjp{h}", tag="adjp")
            src = d["adjT"][h].rearrange("(jt p) i -> p jt i", p=128)
            for ck in range(2):
                lo, hi_ = 4 * ck, 4 * ck + 4
                adjt = adjpool.tile([128, 4, 1024], DT.float32, name=f"adjt{h}{ck}", tag="adj")
                nc.sync.dma_start(out=adjt[:, :, :], in_=src[:, lo:hi_, :])
                a_hi = min(hi_, NJA)
                if a_hi > lo:
                    nc.scalar.activation(adjb[:, lo:a_hi, :], adjt[:, 0:a_hi - lo, :], AF.Exp)
                p_lo = max(lo, NJA)
                if hi_ > p_lo:
                    nc.vector.tensor_scalar(adjp[:, p_lo - NJA:hi_ - NJA, :],
                                            adjt[:, p_lo - lo:hi_ - lo, :],
                                            C1, BCONST, ALU.mult, ALU.add)
            adjbs.append(adjb)
            adjps.append(adjp)
        for bl in range(BL):
            # Zero-init each pav bank with a k=1 dummy matmul (start=True sets
            # has_written for all partitions) so the per-band accumulations can
            # all use start=False in any order. (Verified necessary on HW: a
            # lone start=True first-write gets reordered and corrupts.)
            pavs = []
            for ih in range(2):
                pav = ppool2.tile([128, 512], DT.float32, name=f"pav{hp}{bl}{ih}", tag="p2")
                nc.tensor.matmul(pav[:, :], ones_row[:, :], zrow[:, :],
                                 start=True, stop=False, skip_group_check=True)
                pavs.append(pav)
            # Software pipeline: attn@v matmuls are issued one jt late so the
            # PE's strict FIFO never stalls waiting on the exp of the current
            # tile — the next tile's scores matmuls provide the latency buffer.
            def emit_av(jt, at_mms):
                for hi, h in enumerate((h0, h0 + 1)):
                    g = h % 4
                    for ih in range(2):
                        i_sl = slice(ih * 512, (ih + 1) * 512)
                        nc.tensor.matmul(pavs[ih][32 * g:32 * g + 17, :],
                                         v_aug[:, bl, h, jt, :], at_mms[hi][:, i_sl],
                                         start=False, stop=(jt == 7),
                                         tile_position=(0, 32 * g),
                                         skip_group_check=True)

            flush_norm()
            pending = []
            for jt in range(8):
                k_sl = slice(bl * N + jt * 128, bl * N + (jt + 1) * 128)
                pss = [pspool.tile([128, 1024], DT.float32, name=f"ps{h0 + hi}{bl}{jt}", tag="ps")
                       for hi in range(2)]
                # scores first (adjacent different-band matmuls overlap in the
                # PE and their LDWEIGHTS pull ahead), then the full-array
                # identity matmuls that accumulate adj on top
                for hi, h in enumerate((h0, h0 + 1)):
                    g, hg = h % 4, h // 4
                    for ih in range(2):
                        i_sl = slice(ih * 512, (ih + 1) * 512)
                        q_sl = slice(bl * N + ih * 512, bl * N + (ih + 1) * 512)
                        nc.tensor.matmul(pss[hi][:, i_sl],
                                         k_band[32 * g:32 * g + 16, hg, k_sl],
                                         q_band[32 * g:32 * g + 16, hg, q_sl],
                                         start=True, stop=True,
                                         tile_position=(32 * g, 0))
                if len(pending) > 2:
                    emit_av(*pending.pop(0))
                at_mms = []
                for hi, h in enumerate((h0, h0 + 1)):
                    if jt < NJA:
                        ate = attnpool.tile([128, 1024], DT.bfloat16, name=f"ate{h}{bl}{jt}", tag="ate", bufs=3)
                        nc.scalar.activation(ate[:, :], pss[hi][:, :], AF.Exp)
                        at = attnpool.tile([128, 1024], DT.bfloat16, name=f"at{h}{bl}{jt}", tag="at")
                        nc.vector.scalar_tensor_tensor(at[:, :], ate[:, :], 1.0,
                                                       adjbs[hi][:, jt, :],
                                                       ALU.bypass, ALU.mult)
                        at_mms.append(at)
                    else:
                        ati = attnpool.tile([128, 1024], DT.int16, name=f"ati{h}{bl}{jt}", tag="ati")
                        nc.vector.scalar_tensor_tensor(ati[:, :], pss[hi][:, :], C1,
                                                       adjps[hi][:, jt - NJA, :],
                                                       ALU.mult, ALU.add)
                        at_mms.append(ati.bitcast(DT.bfloat16))
                pending.append((jt, at_mms))
            for pe_ in pending:
                emit_av(*pe_)
            # normalization: rec = 1/denominator via exp(-ln(|x|+eps)) on
            # ScalarE, then one k=128 selector matmul broadcasts each band's
            # denominator row to the whole band. ACT ops grouped by table set.
            stashes = []
            for ih in range(2):
                stash = smallpool.tile([128, 512], DT.bfloat16, name=f"stash{ih}", tag=f"stash{ih}", bufs=4)
                nc.scalar.copy(stash[:, :], pavs[ih][:, :])
                stashes.append(stash)
            absbs, lnbs, recs = [], [], []
            for ih in range(2):
                absb = smallpool.tile([128, 512], DT.float32, name=f"absb{ih}", tag=f"absb{ih}", bufs=2)
                nc.scalar.activation(absb[:, :], stashes[ih][:, :], AF.Abs)
                absbs.append(absb)
            for ih in range(2):
                lnb = smallpool.tile([128, 512], DT.float32, name=f"lnb{ih}", tag=f"lnb{ih}", bufs=2)
                nc.scalar.activation(lnb[:, :], absbs[ih][:, :], AF.Ln, bias=eps_c[:, :])
                lnbs.append(lnb)
            for ih in range(2):
                rec = smallpool.tile([128, 512], DT.bfloat16, name=f"rec{ih}", tag=f"rec{ih}", bufs=4)
                nc.scalar.activation(rec[:, :], lnbs[ih][:, :], AF.Exp, scale=-1.0)
                recs.append(rec)
            deferred_norms.append((h0, bl, recs, stashes))

    while deferred_norms:
        flush_norm(depth=0)

    # ---- output projection --------------------------------------------------
    out_sb = cpool.tile([128, NI], DT.float32)
    for t in range(NI // 512):
        sl = slice(t * 512, (t + 1) * 512)
        po = ppool2.tile([128, 512], DT.float32, name=f"po{t}", tag="p2")
        nc.tensor.matmul(po[:, :], woT_bf[:, :], avT[:, sl], start=True, stop=True)
        nc.scalar.activation(out_sb[:, sl], po[:, :], AF.Identity, bias=bo_t[:, :])
    nc.sync.dma_start(out=d["outT"][:, :], in_=out_sb[:, :])


def _patch_act_tables():
    """Force Exp/Ln/Identity to resolve to the one table set containing all of
    them, so interleaved exp (softmax) and ln/exp (reciprocal) don't thrash
    ACT_TABLE_LOADs (~2.7us each)."""
    if getattr(bacc, "_act_tables_patched", False):
        return
    orig = bacc.get_activation_tables

    def patched(arch):
        tabs = {k: set(v) for k, v in orig(arch).items()}
        want = {AF.Exp, AF.Ln, AF.Identity, AF.Square}
        combined = None
        for name, funcs in tabs.items():
            if want <= funcs:
                combined = name
                break
        if combined is not None:
            for name, funcs in tabs.items():
                if name != combined:
                    funcs -= want
        return tabs

    bacc.get_activation_tables = patched
    bacc._act_tables_patched = True


def _patch_ldw_opt():
    if getattr(bass_utils, "_ldw_patched", False):
        return
    orig = bass_utils.run_command

    def patched(argv, **kw):
        argv = ["--enable-ldw-opt=true" if a == "--enable-ldw-opt=false" else a
                for a in argv]
        return orig(argv, **kw)

    bass_utils.run_command = patched
    bass_utils._ldw_patched = True


def _build():
    if os.environ.get("KERNEL_LDW_OPT", "0") == "1":
        _patch_ldw_opt()
    nc = bacc.Bacc("TRN2", target_bir_lowering=False, debug=False, num_devices=N_CORES)
    io = {}
    io["xT"] = nc.dram_tensor("xT", [128, NI], DT.float32, kind="ExternalInput").ap()
    io["adjT"] = nc.dram_tensor("adjT", [HEADS, N, N], DT.float32, kind="ExternalInput").ap()
    io["wqk"] = nc.dram_tensor("wqk", [128, 256], DT.float32, kind="ExternalInput").ap()
    io["wvT"] = nc.dram_tensor("wvT", [128, 128], DT.float32, kind="ExternalInput").ap()
    io["woT"] = nc.dram_tensor("woT", [128, 128], DT.float32, kind="ExternalInput").ap()
    io["bq2"] = nc.dram_tensor("bq2", [128, 1], DT.float32, kind="ExternalInput").ap()
    io["bk2"] = nc.dram_tensor("bk2", [128, 1], DT.float32, kind="ExternalInput").ap()
    io["bo2"] = nc.dram_tensor("bo2", [128, 1], DT.float32, kind="ExternalInput").ap()
    io["bvr"] = nc.dram_tensor("bvr", [1, 128], DT.float32, kind="ExternalInput").ap()
    io["esel"] = nc.dram_tensor("esel", [128, 128], DT.float32, kind="ExternalInput").ap()
    io["outT"] = nc.dram_tensor("outT", [128, NI], DT.float32, kind="ExternalOutput").ap()
    nc._io_aps = io
    import contextlib
    with tile.TileContext(nc) as tc:
        with contextlib.ExitStack() as stack:
            _emit(nc, tc, stack)
    nc.compile()
    return nc


def kernel(x, Wq, bq, Wk, bk, Wv, bv, Wo, bo, adj):
    global _CACHED_NC, LAST_EXEC_NS
    x = np.asarray(x, np.float32)
    Wq, Wk, Wv, Wo = (np.asarray(w, np.float32) for w in (Wq, Wk, Wv, Wo))
    bq, bk, bv, bo = (np.asarray(b_, np.float32) for b_ in (bq, bk, bv, bo))
    adj = np.asarray(adj, np.float32)

    if _CACHED_NC is None:
        _CACHED_NC = _build()
    nc = _CACHED_NC

    adjT = np.ascontiguousarray(adj[0].transpose(0, 2, 1))      # (8, N, N), [h, j, i]
    wqk_np = np.ascontiguousarray(np.concatenate([Wq.T, Wk.T], axis=1))  # (128, 256)
    wvT_np = np.ascontiguousarray(Wv.T)
    woT_np = np.ascontiguousarray(Wo.T)
    shared = {
        "adjT": adjT,
        "wqk": wqk_np,
        "wvT": wvT_np,
        "woT": woT_np,
        "bq2": np.ascontiguousarray(bq.reshape(128, 1)),
        "bk2": np.ascontiguousarray(bk.reshape(128, 1)),
        "bo2": np.ascontiguousarray(bo.reshape(128, 1)),
        "bvr": np.ascontiguousarray(bv.reshape(1, 128)),
        "esel": np.ascontiguousarray(
            (np.arange(128)[:, None] == (32 * (np.arange(128)[None, :] // 32) + 16)
             ).astype(np.float32)),
    }
    in_maps = []
    for c in range(N_CORES):
        xT_c = np.ascontiguousarray(x[BL * c:BL * (c + 1)].reshape(NI, H).T)
        m = dict(shared)
        m["xT"] = xT_c
        in_maps.append(m)

    trace = os.environ.get("KERNEL_TRACE", "0") == "1"
    if trace:
        _install_ntff_hook()
    res = bass_utils.run_bass_kernel_spmd(nc, in_maps, core_ids=list(range(N_CORES)), trace=trace)
    LAST_EXEC_NS = res.exec_time_ns

    out = np.empty((B, N, H), np.float32)
    for c in range(N_CORES):
        oT = res.results[c]["outT"]
        for bl in range(BL):
            out[BL * c + bl] = oT[:, bl * N:(bl + 1) * N].T
    # softmax rows sum to 1 exactly => mean(|attn|) == 1/N (attn > 0)
    loss = np.float32(1e-4 / N)
    return out, loss


# revision 71
# speedup vs baseline: 1.0402x; 1.0402x over previous
"""Trainium2 Bass kernel for AdaptiveGraphAttentionModule.

Model (per reference):
  q,k,v = linear projections of x, split into 8 heads of dim 16
  scores = q@k^T * (1/4) + adj          (adj broadcast over batch)
  attn   = softmax(scores, axis=-1)
  out    = (attn @ v) -> merge heads -> linear (Wo, bo)
  loss   = 1e-4 * mean(|attn|)          (== 1e-4/N exactly: softmax rows sum to 1)

Distribution: data-parallel over batch. 8 cores x 2 batches each; weights and
adj replicated. No collectives.

Device-side design (per core; "T" = feature-on-partitions transposed):
  - scores are computed transposed (scoresT[j,i] = sum_d k[j,d] q[i,d]) so the
    resulting attn (key index j on partitions) streams directly into the
    attn@v matmul with no transpose of the big N*N tensor.
  - heads are processed in pairs on adjacent 32-partition bands so their K=16
    matmuls can overlap in the PE array (32-row tile_position groups).
  - per (head, batch), jt key-tiles 0..NJA-1 take the exact route: exp(s) on
    ScalarE, then a bf16 VectorE multiply by E = exp(adj) (precomputed once
    per head on ScalarE while the adj tile is staged, exploiting
    exp(s+a) = exp(s)*exp(a)). Tiles NJA..7 take the cheap route: one VectorE
    op computes
    round(scores*128*log2(e) + (adj*128*log2(e) + 16250.5)) as int16, whose
    bits reinterpreted as bf16 are 2^((s+adj)*log2e) = exp(s+adj) to ~3%
    (Schraudolph), folding adj-add + exp + bf16-cast into a single pass.
  - the softmax denominator comes free from an extra ones-column in V (the
    same matmuls that compute attn@v also accumulate the row sums); bv is
    folded into V via a rank-1 accumulating matmul so the normalized output
    picks it up exactly. 1/denominator runs on ScalarE as exp(-ln(|x|+eps))
    and is broadcast to each band by one k=128 selector matmul.
  - attn@v and the normalization consumers are software-pipelined one step
    behind their producers so the PE's strict FIFO never waits on the
    ScalarE/VectorE round trips.
"""

import os
import sys

import numpy as np

for _p in ("/opt/trn_rl_repo",):
    if os.path.isdir(_p) and _p not in sys.path:
        sys.path.insert(0, _p)

import concourse.bacc as bacc
import concourse.mybir as mybir
from concourse import bass_utils, tile

AF = mybir.ActivationFunctionType
ALU = mybir.AluOpType
DT = mybir.dt

N_CORES = 8
B, N, H = 16, 1024, 128
HEADS, HD = 8, 16
BL = B // N_CORES          # batches per core
NI = BL * N                # 2048 tokens per core
SCALE = 1.0 / 4.0          # 1/sqrt(HD)

LAST_EXEC_NS = None
_CACHED_NC = None
_HOOK_DONE = False


def _install_ntff_hook():
    """The agent image lacks ``antenv.axon_hooks``; recreate it so
    run_bass_kernel_spmd(trace=True) can reach the NTFF profiler."""
    global _HOOK_DONE
    if _HOOK_DONE:
        return
    _HOOK_DONE = True
    import types

    try:
        from antenv.axon_hooks import get_axon_ntff_profile_hook  # noqa: F401
    except ImportError:
        import antenv

        mod = types.ModuleType("antenv.axon_hooks")
        _h = {"hook": None}
        mod.set_axon_ntff_profile_hook = lambda h: _h.__setitem__("hook", h)
        mod.get_axon_ntff_profile_hook = lambda: _h["hook"]
        sys.modules["antenv.axon_hooks"] = mod
        antenv.axon_hooks = mod
    from antenv.axon_hooks import (
        get_axon_ntff_profile_hook,
        set_axon_ntff_profile_hook,
    )

    if get_axon_ntff_profile_hook() is None:
        try:
            from trn_agent_boot.trn_boot import _ntff_profile_via_ctypes

            set_axon_ntff_profile_hook(_ntff_profile_via_ctypes("/opt/axon/libaxon_pjrt.so"))
        except Exception:
            pass
    # artifact upload needs bucket credentials the container lacks
    bass_utils.upload_artifacts = lambda tmpdir: tmpdir


def _emit(nc, tc, stack):
    d = nc._io_aps

    cpool = stack.enter_context(tc.tile_pool(name="cpool", bufs=1))
    adjpool = stack.enter_context(tc.tile_pool(name="adjpool", bufs=2))
    adjbfpool = stack.enter_context(tc.tile_pool(name="adjbfpool", bufs=2))
    adjppool = stack.enter_context(tc.tile_pool(name="adjppool", bufs=2))
    attnpool = stack.enter_context(tc.tile_pool(name="attnpool", bufs=8))
    smallpool = stack.enter_context(tc.tile_pool(name="smallpool", bufs=3))
    pspool = stack.enter_context(tc.tile_pool(name="pspool", bufs=2, space="PSUM"))
    ppool2 = stack.enter_context(tc.tile_pool(name="ppool2", bufs=4, space="PSUM"))

    # ---- persistent inputs -------------------------------------------------
    xT = cpool.tile([128, NI], DT.float32)
    nc.sync.dma_start(out=xT[:, :], in_=d["xT"][:, :])
    wqk = cpool.tile([128, 256], DT.float32)
    nc.sync.dma_start(out=wqk[:, :], in_=d["wqk"][:, :])
    wvT = cpool.tile([128, 128], DT.float32)
    nc.sync.dma_start(out=wvT[:, :], in_=d["wvT"][:, :])
    woT = cpool.tile([128, 128], DT.float32)
    nc.sync.dma_start(out=woT[:, :], in_=d["woT"][:, :])
    bq_t = cpool.tile([128, 1], DT.float32)
    nc.sync.dma_start(out=bq_t[:, :], in_=d["bq2"][:, :])
    bk_t = cpool.tile([128, 1], DT.float32)
    nc.sync.dma_start(out=bk_t[:, :], in_=d["bk2"][:, :])
    bo_t = cpool.tile([128, 1], DT.float32)
    nc.sync.dma_start(out=bo_t[:, :], in_=d["bo2"][:, :])
    bv_t = cpool.tile([1, 128], DT.float32)
    nc.sync.dma_start(out=bv_t[:, :], in_=d["bvr"][:, :])

    woT_bf = cpool.tile([128, 128], DT.bfloat16)
    nc.vector.tensor_copy(woT_bf[:, :], woT[:, :])
    bqs = cpool.tile([128, 1], DT.float32)
    nc.vector.tensor_scalar_mul(bqs[:, :], bq_t[:, :], SCALE)

    # bf16 copies of everything that streams through the PE (fp32 matmuls run
    # LOW_HIGH dual-pass ~4x slower)
    xT_bf = cpool.tile([128, NI], DT.bfloat16)
    nc.vector.tensor_copy(xT_bf[:, :], xT[:, :])
    wqk_bf = cpool.tile([128, 256], DT.bfloat16)
    nc.vector.tensor_copy(wqk_bf[:, :], wqk[:, :])
    wvT_bf = cpool.tile([128, 128], DT.bfloat16)
    nc.vector.tensor_copy(wvT_bf[:, :], wvT[:, :])
    bv_bf = cpool.tile([1, 128], DT.bfloat16)
    nc.vector.tensor_copy(bv_bf[:, :], bv_t[:, :])

    ones_row = cpool.tile([1, 128], DT.bfloat16)
    nc.vector.memset(ones_row[:, :], 1.0)
    eps_c = cpool.tile([128, 1], DT.float32)
    nc.vector.memset(eps_c[:, :], 1e-30)
    zrow = cpool.tile([1, 512], DT.bfloat16)
    nc.vector.memset(zrow[:, :], 0.0)
    # denominator broadcast selector: e_sel2[k, m] = 1 iff k == 32*(m//32)+16,
    # i.e. one k=128 matmul copies each band's denominator row to all 32 rows
    # of that band (junk rec rows are multiplied by 0.0, and rec is built
    # NaN-free below so 0*junk == 0).
    esel_f = cpool.tile([128, 128], DT.float32)
    nc.sync.dma_start(out=esel_f[:, :], in_=d["esel"][:, :])
    e_sel = cpool.tile([128, 128], DT.bfloat16)
    nc.vector.tensor_copy(e_sel[:, :], esel_f[:, :])

    # ---- q/k projections ---------------------------------------------------
    # q_lin/k_lin: (feature h*16+d on partitions, token on free), bf16.
    # q is pre-scaled by 1/4.
    q_lin = cpool.tile([128, NI], DT.bfloat16)
    k_lin = cpool.tile([128, NI], DT.bfloat16)
    for t in range(NI // 512):
        sl = slice(t * 512, (t + 1) * 512)
        pq = ppool2.tile([128, 512], DT.float32, name=f"pq{t}", tag="p2")
        nc.tensor.matmul(pq[:, :], wqk_bf[:, 0:128], xT_bf[:, sl], start=True, stop=True)
        nc.scalar.activation(q_lin[:, sl], pq[:, :], AF.Identity, bias=bqs[:, :], scale=SCALE)
        pk = ppool2.tile([128, 512], DT.float32, name=f"pk{t}", tag="p2")
        nc.tensor.matmul(pk[:, :], wqk_bf[:, 128:256], xT_bf[:, sl], start=True, stop=True)
        nc.scalar.activation(k_lin[:, sl], pk[:, :], AF.Identity, bias=bk_t[:, :])

    # Rearrange q/k so head h sits at a 32-aligned partition base:
    # band g = h%4 holds heads {g, g+4} (disambiguated along free axis).
    q_band = cpool.tile([128, 2, NI], DT.bfloat16)
    k_band = cpool.tile([128, 2, NI], DT.bfloat16)
    for h in range(HEADS):
        g, hg = h % 4, h // 4
        nc.sync.dma_start(out=q_band[32 * g:32 * g + 16, hg, :], in_=q_lin[h * 16:(h + 1) * 16, :])
        nc.sync.dma_start(out=k_band[32 * g:32 * g + 16, hg, :], in_=k_lin[h * 16:(h + 1) * 16, :])

    # ---- v (natural layout: key index j on partitions) + ones column -------
    # v_aug[p, bl, h, jt, 0:16] = v rows, [..., 16] = 1.0 (softmax denominator)
    v_aug = cpool.tile([128, BL, HEADS, 8, 17], DT.bfloat16)
    nc.vector.memset(v_aug[:, :, :, :, :], 1.0)
    for bl in range(BL):
        for jt in range(8):
            pv = ppool2.tile([128, 512], DT.float32, name=f"pv{bl}{jt}", tag="p2")
            c0 = bl * N + jt * 128
            nc.tensor.matmul(pv[:, 0:128], xT_bf[:, c0:c0 + 128], wvT_bf[:, :], start=True, stop=False)
            nc.tensor.matmul(pv[:, 0:128], ones_row[:, :], bv_bf[:, :], start=False, stop=True)
            nc.vector.tensor_copy(
                v_aug[:, bl, :, jt, 0:16],
                pv[:, 0:128].rearrange("p (h d) -> p h d", h=HEADS),
            )

    # ---- attention main loop ------------------------------------------------

    # Main loop over head pairs: heads (2p, 2p+1) sit on adjacent 32-partition
    # bands so their K=16 scores matmuls can overlap in the PE array and their
    # attn@v matmuls go to adjacent PSUM col-bands of one shared tile.
    # Per (head, batch), jt tiles 0..NJA-1 take the exact route (adj added via
    # identity matmul, exp on ScalarE); tiles NJA..7 take the cheap route
    # (adj + exp2 bit-trick fused into one VectorE op producing bf16 bits).
    NJA = 4
    C1 = 128.0 * 1.4426950408889634          # 128*log2(e)
    BCONST = 16256.0 - 5.513                 # bf16 exponent bias - minimax shift
    avT = cpool.tile([128, NI], DT.bfloat16)
    deferred_norms = []

    def flush_norm(depth=2):
        # deferred two iterations so the bcast matmul never waits in the PE
        # FIFO for the ACT reciprocal chain (ACT's own queue delays it)
        if len(deferred_norms) <= depth:
            return
        fh0, fbl, frecs, fstashes = deferred_norms.pop(0)
        for ih in range(2):
            pbc = ppool2.tile([128, 512], DT.float32, name=f"pbc{fh0}{fbl}{ih}", tag="p2")
            nc.tensor.matmul(pbc[:, :], e_sel[:, :], frecs[ih][:, :], start=True, stop=True)
            bcsb = smallpool.tile([128, 512], DT.bfloat16, name="bcsb", tag="bcsb")
            nc.vector.tensor_copy(bcsb[:, :], pbc[:, :])
            tmp = smallpool.tile([128, 512], DT.bfloat16, name="tmp", tag="tmp")
            q_sl = slice(fbl * N + ih * 512, fbl * N + (ih + 1) * 512)
            for h in (fh0, fh0 + 1):
                g = h % 4
                nc.vector.scalar_tensor_tensor(tmp[32 * g:32 * g + 16, :],
                                               fstashes[ih][32 * g:32 * g + 16, :], 1.0,
                                               bcsb[32 * g:32 * g + 16, :],
                                               ALU.bypass, ALU.mult)
                nc.sync.dma_start(out=avT[h * 16:(h + 1) * 16, q_sl],
                                  in_=tmp[32 * g:32 * g + 16, :])

    for hp in range(HEADS // 2):
        h0 = 2 * hp
        adjbs, adjps = [], []
        for h in (h0, h0 + 1):
            adjb = adjbfpool.tile([128, NJA, 1024], DT.bfloat16, name=f"adjb{h}", tag="adjb")
            adjp = adjppool.tile([128, 8 - NJA, 1024], DT.int16, name=f"adjp{h}", tag="adjp")
            src = d["adjT"][h].rearrange("(jt p) i -> p jt i", p=128)
            for ck in range(2):
                lo, hi_ = 4 * ck, 4 * ck + 4
                adjt = adjpool.tile([128, 4, 1024], DT.float32, name=f"adjt{h}{ck}", tag="adj")
                nc.sync.dma_start(out=adjt[:, :, :], in_=src[:, lo:hi_, :])
                a_hi = min(hi_, NJA)
                if a_hi > lo:
                    nc.scalar.activation(adjb[:, lo:a_hi, :], adjt[:, 0:a_hi - lo, :], AF.Exp)
                p_lo = max(lo, NJA)
                if hi_ > p_lo:
                    nc.vector.tensor_scalar(adjp[:, p_lo - NJA:hi_ - NJA, :],
                                            adjt[:, p_lo - lo:hi_ - lo, :],
                                            C1, BCONST, ALU.mult, ALU.add)
            adjbs.append(adjb)
            adjps.append(adjp)
        for bl in range(BL):
            # Zero-init each pav bank with a k=1 dummy matmul (start=True sets
            # has_written for all partitions) so the per-band accumulations can
            # all use start=False in any order. (Verified necessary on HW: a
            # lone start=True first-write gets reordered and corrupts.)
            pavs = []
            for ih in range(2):
                pav = ppool2.tile([128, 512], DT.float32, name=f"pav{hp}{bl}{ih}", tag="p2")
                nc.tensor.matmul(pav[:, :], ones_row[:, :], zrow[:, :],
                                 start=True, stop=False, skip_group_check=True)
                pavs.append(pav)
            # Software pipeline: attn@v matmuls are issued one jt late so the
            # PE's strict FIFO never stalls waiting on the exp of the current
            # tile — the next tile's scores matmuls provide the latency buffer.
            def emit_av(jt, at_mms):
                for hi, h in enumerate((h0, h0 + 1)):
                    g = h % 4
                    for ih in range(2):
                        i_sl = slice(ih * 512, (ih + 1) * 512)
                        nc.tensor.matmul(pavs[ih][32 * g:32 * g + 17, :],
                                         v_aug[:, bl, h, jt, :], at_mms[hi][:, i_sl],
                                         start=False, stop=(jt == 7),
                                         tile_position=(0, 32 * g),
                                         skip_group_check=True)

            flush_norm()
            pending = []
            for jt in range(8):
                k_sl = slice(bl * N + jt * 128, bl * N + (jt + 1) * 128)
                pss = [pspool.tile([128, 1024], DT.float32, name=f"ps{h0 + hi}{bl}{jt}", tag="ps")
                       for hi in range(2)]
                # scores first (adjacent different-band matmuls overlap in the
                # PE and their LDWEIGHTS pull ahead), then the full-array
                # identity matmuls that accumulate adj on top
                for hi, h in enumerate((h0, h0 + 1)):
                    g, hg = h % 4, h // 4
                    for ih in range(2):
                        i_sl = slice(ih * 512, (ih + 1) * 512)
                        q_sl = slice(bl * N + ih * 512, bl * N + (ih + 1) * 512)
                        nc.tensor.matmul(pss[hi][:, i_sl],
                                         k_band[32 * g:32 * g + 16, hg, k_sl],
                                         q_band[32 * g:32 * g + 16, hg, q_sl],
                                         start=True, stop=True,
                                         tile_position=(32 * g, 0))
                if len(pending) > 2:
                    emit_av(*pending.pop(0))
                at_mms = []
                for hi, h in enumerate((h0, h0 + 1)):
                    if jt < NJA:
                        ate = attnpool.tile([128, 1024], DT.bfloat16, name=f"ate{h}{bl}{jt}", tag="ate", bufs=3)
                        nc.scalar.activation(ate[:, :], pss[hi][:, :], AF.Exp)
                        at = attnpool.tile([128, 1024], DT.bfloat16, name=f"at{h}{bl}{jt}", tag="at")
                        nc.vector.scalar_tensor_tensor(at[:, :], ate[:, :], 1.0,
                                                       adjbs[hi][:, jt, :],
                                                       ALU.bypass, ALU.mult)
                        at_mms.append(at)
                    else:
                        ati = attnpool.tile([128, 1024], DT.int16, name=f"ati{h}{bl}{jt}", tag="ati")
                        nc.vector.scalar_tensor_tensor(ati[:, :], pss[hi][:, :], C1,
                                                       adjps[hi][:, jt - NJA, :],
                                                       ALU.mult, ALU.add)
                        at_mms.append(ati.bitcast(DT.bfloat16))
                pending.append((jt, at_mms))
            for pe_ in pending:
                emit_av(*pe_)
            # normalization: rec = 1/denominator via exp(-ln(|x|+eps)) on
            # ScalarE, then one k=128 selector matmul broadcasts each band's
            # denominator row to the whole band. ACT ops grouped by table set.
            stashes = []
            for ih in range(2):
                stash = smallpool.tile([128, 512], DT.bfloat16, name=f"stash{ih}", tag=f"stash{ih}", bufs=4)
                nc.scalar.copy(stash[:, :], pavs[ih][:, :])
                stashes.append(stash)
            absbs, lnbs, recs = [], [], []
            for ih in range(2):
                absb = smallpool.tile([128, 512], DT.float32, name=f"absb{ih}", tag=f"absb{ih}", bufs=2)
                nc.scalar.activation(absb[:, :], stashes[ih][:, :], AF.Abs)
                absbs.append(absb)
            for ih in range(2):
                lnb = smallpool.tile([128, 512], DT.float32, name=f"lnb{ih}", tag=f"lnb{ih}", bufs=2)
                nc.scalar.activation(lnb[:, :], absbs[ih][:, :], AF.Ln, bias=eps_c[:, :])
                lnbs.append(lnb)
            for ih in range(2):
                rec = smallpool.tile([128, 512], DT.bfloat16, name=f"rec{ih}", tag=f"rec{ih}", bufs=4)
                nc.scalar.activation(rec[:, :], lnbs[ih][:, :], AF.Exp, scale=-1.0)
                recs.append(rec)
            deferred_norms.append((h0, bl, recs, stashes))

    while deferred_norms:
        flush_norm(depth=0)

    # ---- output projection --------------------------------------------------
    out_sb = cpool.tile([128, NI], DT.float32)
    for t in range(NI // 512):
        sl = slice(t * 512, (t + 1) * 512)
        po = ppool2.tile([128, 512], DT.float32, name=f"po{t}", tag="p2")
        nc.tensor.matmul(po[:, :], woT_bf[:, :], avT[:, sl], start=True, stop=True)
        nc.scalar.activation(out_sb[:, sl], po[:, :], AF.Identity, bias=bo_t[:, :])
    nc.sync.dma_start(out=d["outT"][:, :], in_=out_sb[:, :])


def _patch_act_tables():
    """Force Exp/Ln/Identity to resolve to the one table set containing all of
    them, so interleaved exp (softmax) and ln/exp (reciprocal) don't thrash
    ACT_TABLE_LOADs (~2.7us each)."""
    if getattr(bacc, "_act_tables_patched", False):
        return
    orig = bacc.get_activation_tables

    def patched(arch):
        tabs = {k: set(v) for k, v in orig(arch).items()}
        want = {AF.Exp, AF.Ln, AF.Identity, AF.Square}
        combined = None
        for name, funcs in tabs.items():
            if want <= funcs:
                combined = name
                break
        if combined is not None:
            for name, funcs in tabs.items():
                if name != combined:
                    funcs -= want
        return tabs

    bacc.get_activation_tables = patched
    bacc._act_tables_patched = True


def _patch_ldw_opt():
    if getattr(bass_utils, "_ldw_patched", False):
        return
    orig = bass_utils.run_command

    def patched(argv, **kw):
        argv = ["--enable-ldw-opt=true" if a == "--enable-ldw-opt=false" else a
                for a in argv]
        return orig(argv, **kw)

    bass_utils.run_command = patched
    bass_utils._ldw_patched = True


def _build():
    if os.environ.get("KERNEL_LDW_OPT", "0") == "1":
        _patch_ldw_opt()
    nc = bacc.Bacc("TRN2", target_bir_lowering=False, debug=False, num_devices=N_CORES)
    io = {}
    io["xT"] = nc.dram_tensor("xT", [128, NI], DT.float32, kind="ExternalInput").ap()
    io["adjT"] = nc.dram_tensor("adjT", [HEADS, N, N], DT.float32, kind="ExternalInput").ap()
    io["wqk"] = nc.dram_tensor("wqk", [128, 256], DT.float32, kind="ExternalInput").ap()
    io["wvT"] = nc.dram_tensor("wvT", [128, 128], DT.float32, kind="ExternalInput").ap()
    io["woT"] = nc.dram_tensor("woT", [128, 128], DT.float32, kind="ExternalInput").ap()
    io["bq2"] = nc.dram_tensor("bq2", [128, 1], DT.float32, kind="ExternalInput").ap()
    io["bk2"] = nc.dram_tensor("bk2", [128, 1], DT.float32, kind="ExternalInput").ap()
    io["bo2"] = nc.dram_tensor("bo2", [128, 1], DT.float32, kind="ExternalInput").ap()
    io["bvr"] = nc.dram_tensor("bvr", [1, 128], DT.float32, kind="ExternalInput").ap()
    io["esel"] = nc.dram_tensor("esel", [128, 128], DT.float32, kind="ExternalInput").ap()
    io["outT"] = nc.dram_tensor("outT", [128, NI], DT.float32, kind="ExternalOutput").ap()
    nc._io_aps = io
    import contextlib
    with tile.TileContext(nc) as tc:
        with contextlib.ExitStack() as stack:
            _emit(nc, tc, stack)
    nc.compile()
    return nc


def kernel(x, Wq, bq, Wk, bk, Wv, bv, Wo, bo, adj):
    global _CACHED_NC, LAST_EXEC_NS
    x = np.asarray(x, np.float32)
    Wq, Wk, Wv, Wo = (np.asarray(w, np.float32) for w in (Wq, Wk, Wv, Wo))
    bq, bk, bv, bo = (np.asarray(b_, np.float32) for b_ in (bq, bk, bv, bo))
    adj = np.asarray(adj, np.float32)

    if _CACHED_NC is None:
        _CACHED_NC = _build()
    nc = _CACHED_NC

    adjT = np.ascontiguousarray(adj[0].transpose(0, 2, 1))      # (8, N, N), [h, j, i]
    wqk_np = np.ascontiguousarray(np.concatenate([Wq.T, Wk.T], axis=1))  # (128, 256)
    wvT_np = np.ascontiguousarray(Wv.T)
    woT_np = np.ascontiguousarray(Wo.T)
    shared = {
        "adjT": adjT,
        "wqk": wqk_np,
        "wvT": wvT_np,
        "woT": woT_np,
        "bq2": np.ascontiguousarray(bq.reshape(128, 1)),
        "bk2": np.ascontiguousarray(bk.reshape(128, 1)),
        "bo2": np.ascontiguousarray(bo.reshape(128, 1)),
        "bvr": np.ascontiguousarray(bv.reshape(1, 128)),
        "esel": np.ascontiguousarray(
            (np.arange(128)[:, None] == (32 * (np.arange(128)[None, :] // 32) + 16)
             ).astype(np.float32)),
    }
    in_maps = []
    for c in range(N_CORES):
        xT_c = np.ascontiguousarray(x[BL * c:BL * (c + 1)].reshape(NI, H).T)
        m = dict(shared)
        m["xT"] = xT_c
        in_maps.append(m)

    trace = os.environ.get("KERNEL_TRACE", "0") == "1"
    if trace:
        _install_ntff_hook()
    res = bass_utils.run_bass_kernel_spmd(nc, in_maps, core_ids=list(range(N_CORES)), trace=trace)
    LAST_EXEC_NS = res.exec_time_ns

    out = np.empty((B, N, H), np.float32)
    for c in range(N_CORES):
        oT = res.results[c]["outT"]
        for bl in range(BL):
            out[BL * c + bl] = oT[:, bl * N:(bl + 1) * N].T
    # softmax rows sum to 1 exactly => mean(|attn|) == 1/N (attn > 0)
    loss = np.float32(1e-4 / N)
    return out, loss


# revision 73
# speedup vs baseline: 1.0584x; 1.0175x over previous
"""Trainium2 Bass kernel for AdaptiveGraphAttentionModule.

Model (per reference):
  q,k,v = linear projections of x, split into 8 heads of dim 16
  scores = q@k^T * (1/4) + adj          (adj broadcast over batch)
  attn   = softmax(scores, axis=-1)
  out    = (attn @ v) -> merge heads -> linear (Wo, bo)
  loss   = 1e-4 * mean(|attn|)          (== 1e-4/N exactly: softmax rows sum to 1)

Distribution: data-parallel over batch. 8 cores x 2 batches each; weights and
adj replicated. No collectives.

Device-side design (per core; "T" = feature-on-partitions transposed):
  - scores are computed transposed (scoresT[j,i] = sum_d k[j,d] q[i,d]) so the
    resulting attn (key index j on partitions) streams directly into the
    attn@v matmul with no transpose of the big N*N tensor.
  - heads are processed in pairs on adjacent 32-partition bands so their K=16
    matmuls can overlap in the PE array (32-row tile_position groups).
  - per (head, batch), jt key-tiles 0..NJA-1 take the exact route: exp(s) on
    ScalarE, then a bf16 VectorE multiply by E = exp(adj) (precomputed once
    per head on ScalarE while the adj tile is staged, exploiting
    exp(s+a) = exp(s)*exp(a)). Tiles NJA..7 take the cheap route: one VectorE
    op computes
    round(scores*128*log2(e) + (adj*128*log2(e) + 16250.5)) as int16, whose
    bits reinterpreted as bf16 are 2^((s+adj)*log2e) = exp(s+adj) to ~3%
    (Schraudolph), folding adj-add + exp + bf16-cast into a single pass.
  - the softmax denominator comes free from an extra ones-column in V (the
    same matmuls that compute attn@v also accumulate the row sums); bv is
    folded into V via a rank-1 accumulating matmul so the normalized output
    picks it up exactly. 1/denominator runs on ScalarE as exp(-ln(|x|+eps))
    and is broadcast to each band by one k=128 selector matmul.
  - attn@v and the normalization consumers are software-pipelined one step
    behind their producers so the PE's strict FIFO never waits on the
    ScalarE/VectorE round trips.
"""

import os
import sys

import numpy as np

for _p in ("/opt/trn_rl_repo",):
    if os.path.isdir(_p) and _p not in sys.path:
        sys.path.insert(0, _p)

import concourse.bacc as bacc
import concourse.mybir as mybir
from concourse import bass_utils, tile

AF = mybir.ActivationFunctionType
ALU = mybir.AluOpType
DT = mybir.dt

N_CORES = 8
B, N, H = 16, 1024, 128
HEADS, HD = 8, 16
BL = B // N_CORES          # batches per core
NI = BL * N                # 2048 tokens per core
SCALE = 1.0 / 4.0          # 1/sqrt(HD)

LAST_EXEC_NS = None
_CACHED_NC = None
_HOOK_DONE = False


def _install_ntff_hook():
    """The agent image lacks ``antenv.axon_hooks``; recreate it so
    run_bass_kernel_spmd(trace=True) can reach the NTFF profiler."""
    global _HOOK_DONE
    if _HOOK_DONE:
        return
    _HOOK_DONE = True
    import types

    try:
        from antenv.axon_hooks import get_axon_ntff_profile_hook  # noqa: F401
    except ImportError:
        import antenv

        mod = types.ModuleType("antenv.axon_hooks")
        _h = {"hook": None}
        mod.set_axon_ntff_profile_hook = lambda h: _h.__setitem__("hook", h)
        mod.get_axon_ntff_profile_hook = lambda: _h["hook"]
        sys.modules["antenv.axon_hooks"] = mod
        antenv.axon_hooks = mod
    from antenv.axon_hooks import (
        get_axon_ntff_profile_hook,
        set_axon_ntff_profile_hook,
    )

    if get_axon_ntff_profile_hook() is None:
        try:
            from trn_agent_boot.trn_boot import _ntff_profile_via_ctypes

            set_axon_ntff_profile_hook(_ntff_profile_via_ctypes("/opt/axon/libaxon_pjrt.so"))
        except Exception:
            pass
    # artifact upload needs bucket credentials the container lacks
    bass_utils.upload_artifacts = lambda tmpdir: tmpdir


def _emit(nc, tc, stack):
    d = nc._io_aps

    cpool = stack.enter_context(tc.tile_pool(name="cpool", bufs=1))
    adjpool = stack.enter_context(tc.tile_pool(name="adjpool", bufs=2))
    adjbfpool = stack.enter_context(tc.tile_pool(name="adjbfpool", bufs=3))
    adjppool = stack.enter_context(tc.tile_pool(name="adjppool", bufs=2))
    attnpool = stack.enter_context(tc.tile_pool(name="attnpool", bufs=8))
    smallpool = stack.enter_context(tc.tile_pool(name="smallpool", bufs=3))
    pspool = stack.enter_context(tc.tile_pool(name="pspool", bufs=2, space="PSUM"))
    ppool2 = stack.enter_context(tc.tile_pool(name="ppool2", bufs=4, space="PSUM"))

    # ---- persistent inputs -------------------------------------------------
    xT = cpool.tile([128, NI], DT.float32)
    nc.sync.dma_start(out=xT[:, :], in_=d["xT"][:, :])
    wqk = cpool.tile([128, 256], DT.float32)
    nc.sync.dma_start(out=wqk[:, :], in_=d["wqk"][:, :])
    wvT = cpool.tile([128, 128], DT.float32)
    nc.sync.dma_start(out=wvT[:, :], in_=d["wvT"][:, :])
    woT = cpool.tile([128, 128], DT.float32)
    nc.sync.dma_start(out=woT[:, :], in_=d["woT"][:, :])
    bq_t = cpool.tile([128, 1], DT.float32)
    nc.sync.dma_start(out=bq_t[:, :], in_=d["bq2"][:, :])
    bk_t = cpool.tile([128, 1], DT.float32)
    nc.sync.dma_start(out=bk_t[:, :], in_=d["bk2"][:, :])
    bo_t = cpool.tile([128, 1], DT.float32)
    nc.sync.dma_start(out=bo_t[:, :], in_=d["bo2"][:, :])
    bv_t = cpool.tile([1, 128], DT.float32)
    nc.sync.dma_start(out=bv_t[:, :], in_=d["bvr"][:, :])

    woT_bf = cpool.tile([128, 128], DT.bfloat16)
    nc.vector.tensor_copy(woT_bf[:, :], woT[:, :])
    bqs = cpool.tile([128, 1], DT.float32)
    nc.vector.tensor_scalar_mul(bqs[:, :], bq_t[:, :], SCALE)

    # bf16 copies of everything that streams through the PE (fp32 matmuls run
    # LOW_HIGH dual-pass ~4x slower)
    xT_bf = cpool.tile([128, NI], DT.bfloat16)
    nc.vector.tensor_copy(xT_bf[:, :], xT[:, :])
    wqk_bf = cpool.tile([128, 256], DT.bfloat16)
    nc.vector.tensor_copy(wqk_bf[:, :], wqk[:, :])
    wvT_bf = cpool.tile([128, 128], DT.bfloat16)
    nc.vector.tensor_copy(wvT_bf[:, :], wvT[:, :])
    bv_bf = cpool.tile([1, 128], DT.bfloat16)
    nc.vector.tensor_copy(bv_bf[:, :], bv_t[:, :])

    ones_row = cpool.tile([1, 128], DT.bfloat16)
    nc.vector.memset(ones_row[:, :], 1.0)
    eps_c = cpool.tile([128, 1], DT.float32)
    nc.vector.memset(eps_c[:, :], 1e-30)
    zrow = cpool.tile([1, 512], DT.bfloat16)
    nc.vector.memset(zrow[:, :], 0.0)
    # denominator broadcast selector: e_sel2[k, m] = 1 iff k == 32*(m//32)+16,
    # i.e. one k=128 matmul copies each band's denominator row to all 32 rows
    # of that band (junk rec rows are multiplied by 0.0, and rec is built
    # NaN-free below so 0*junk == 0).
    esel_f = cpool.tile([128, 128], DT.float32)
    nc.sync.dma_start(out=esel_f[:, :], in_=d["esel"][:, :])
    e_sel = cpool.tile([128, 128], DT.bfloat16)
    nc.vector.tensor_copy(e_sel[:, :], esel_f[:, :])

    # ---- q/k projections ---------------------------------------------------
    # q_lin/k_lin: (feature h*16+d on partitions, token on free), bf16.
    # q is pre-scaled by 1/4.
    q_lin = cpool.tile([128, NI], DT.bfloat16)
    k_lin = cpool.tile([128, NI], DT.bfloat16)
    for t in range(NI // 512):
        sl = slice(t * 512, (t + 1) * 512)
        pq = ppool2.tile([128, 512], DT.float32, name=f"pq{t}", tag="p2")
        nc.tensor.matmul(pq[:, :], wqk_bf[:, 0:128], xT_bf[:, sl], start=True, stop=True)
        nc.scalar.activation(q_lin[:, sl], pq[:, :], AF.Identity, bias=bqs[:, :], scale=SCALE)
        pk = ppool2.tile([128, 512], DT.float32, name=f"pk{t}", tag="p2")
        nc.tensor.matmul(pk[:, :], wqk_bf[:, 128:256], xT_bf[:, sl], start=True, stop=True)
        nc.scalar.activation(k_lin[:, sl], pk[:, :], AF.Identity, bias=bk_t[:, :])

    # Rearrange q/k so head h sits at a 32-aligned partition base:
    # band g = h%4 holds heads {g, g+4} (disambiguated along free axis).
    q_band = cpool.tile([128, 2, NI], DT.bfloat16)
    k_band = cpool.tile([128, 2, NI], DT.bfloat16)
    for h in range(HEADS):
        g, hg = h % 4, h // 4
        nc.sync.dma_start(out=q_band[32 * g:32 * g + 16, hg, :], in_=q_lin[h * 16:(h + 1) * 16, :])
        nc.sync.dma_start(out=k_band[32 * g:32 * g + 16, hg, :], in_=k_lin[h * 16:(h + 1) * 16, :])

    # ---- v (natural layout: key index j on partitions) + ones column -------
    # v_aug[p, bl, h, jt, 0:16] = v rows, [..., 16] = 1.0 (softmax denominator)
    v_aug = cpool.tile([128, BL, HEADS, 8, 17], DT.bfloat16)
    nc.vector.memset(v_aug[:, :, :, :, :], 1.0)
    for bl in range(BL):
        for jt in range(8):
            pv = ppool2.tile([128, 512], DT.float32, name=f"pv{bl}{jt}", tag="p2")
            c0 = bl * N + jt * 128
            nc.tensor.matmul(pv[:, 0:128], xT_bf[:, c0:c0 + 128], wvT_bf[:, :], start=True, stop=False)
            nc.tensor.matmul(pv[:, 0:128], ones_row[:, :], bv_bf[:, :], start=False, stop=True)
            nc.vector.tensor_copy(
                v_aug[:, bl, :, jt, 0:16],
                pv[:, 0:128].rearrange("p (h d) -> p h d", h=HEADS),
            )

    # ---- attention main loop ------------------------------------------------

    # Main loop over head pairs: heads (2p, 2p+1) sit on adjacent 32-partition
    # bands so their K=16 scores matmuls can overlap in the PE array and their
    # attn@v matmuls go to adjacent PSUM col-bands of one shared tile.
    # Per (head, batch), jt tiles 0..NJA-1 take the exact route (adj added via
    # identity matmul, exp on ScalarE); tiles NJA..7 take the cheap route
    # (adj + exp2 bit-trick fused into one VectorE op producing bf16 bits).
    NJA = 4
    C1 = 128.0 * 1.4426950408889634          # 128*log2(e)
    BCONST = 16256.0 - 5.513                 # bf16 exponent bias - minimax shift
    avT = cpool.tile([128, NI], DT.bfloat16)
    deferred_norms = []

    def flush_norm(depth=2):
        # deferred two iterations so the bcast matmul never waits in the PE
        # FIFO for the ACT reciprocal chain (ACT's own queue delays it)
        if len(deferred_norms) <= depth:
            return
        fh0, fbl, frecs, fstashes = deferred_norms.pop(0)
        for ih in range(2):
            pbc = ppool2.tile([128, 512], DT.float32, name=f"pbc{fh0}{fbl}{ih}", tag="p2")
            nc.tensor.matmul(pbc[:, :], e_sel[:, :], frecs[ih][:, :], start=True, stop=True)
            bcsb = smallpool.tile([128, 512], DT.bfloat16, name="bcsb", tag="bcsb")
            nc.vector.tensor_copy(bcsb[:, :], pbc[:, :])
            tmp = smallpool.tile([128, 512], DT.bfloat16, name="tmp", tag="tmp")
            q_sl = slice(fbl * N + ih * 512, fbl * N + (ih + 1) * 512)
            for h in (fh0, fh0 + 1):
                g = h % 4
                nc.vector.scalar_tensor_tensor(tmp[32 * g:32 * g + 16, :],
                                               fstashes[ih][32 * g:32 * g + 16, :], 1.0,
                                               bcsb[32 * g:32 * g + 16, :],
                                               ALU.bypass, ALU.mult)
                nc.sync.dma_start(out=avT[h * 16:(h + 1) * 16, q_sl],
                                  in_=tmp[32 * g:32 * g + 16, :])

    for hp in range(HEADS // 2):
        h0 = 2 * hp
        adjbs, adjps = [], []
        for h in (h0, h0 + 1):
            adjb = adjbfpool.tile([128, NJA, 1024], DT.bfloat16, name=f"adjb{h}", tag="adjb")
            adjp = adjppool.tile([128, 8 - NJA, 1024], DT.int16, name=f"adjp{h}", tag="adjp")
            src = d["adjT"][h].rearrange("(jt p) i -> p jt i", p=128)
            for ck in range(2):
                lo, hi_ = 4 * ck, 4 * ck + 4
                adjt = adjpool.tile([128, 4, 1024], DT.float32, name=f"adjt{h}{ck}", tag="adj")
                nc.sync.dma_start(out=adjt[:, :, :], in_=src[:, lo:hi_, :])
                a_hi = min(hi_, NJA)
                if a_hi > lo:
                    nc.scalar.activation(adjb[:, lo:a_hi, :], adjt[:, 0:a_hi - lo, :], AF.Exp)
                p_lo = max(lo, NJA)
                if hi_ > p_lo:
                    nc.vector.tensor_scalar(adjp[:, p_lo - NJA:hi_ - NJA, :],
                                            adjt[:, p_lo - lo:hi_ - lo, :],
                                            C1, BCONST, ALU.mult, ALU.add)
            adjbs.append(adjb)
            adjps.append(adjp)
        for bl in range(BL):
            # Zero-init each pav bank with a k=1 dummy matmul (start=True sets
            # has_written for all partitions) so the per-band accumulations can
            # all use start=False in any order. (Verified necessary on HW: a
            # lone start=True first-write gets reordered and corrupts.)
            pavs = []
            for ih in range(2):
                pav = ppool2.tile([128, 512], DT.float32, name=f"pav{hp}{bl}{ih}", tag="p2")
                nc.tensor.matmul(pav[:, :], ones_row[:, :], zrow[:, :],
                                 start=True, stop=False, skip_group_check=True)
                pavs.append(pav)
            # Software pipeline: attn@v matmuls are issued one jt late so the
            # PE's strict FIFO never stalls waiting on the exp of the current
            # tile — the next tile's scores matmuls provide the latency buffer.
            def emit_av(jt, at_mms):
                for hi, h in enumerate((h0, h0 + 1)):
                    g = h % 4
                    for ih in range(2):
                        i_sl = slice(ih * 512, (ih + 1) * 512)
                        nc.tensor.matmul(pavs[ih][32 * g:32 * g + 17, :],
                                         v_aug[:, bl, h, jt, :], at_mms[hi][:, i_sl],
                                         start=False, stop=(jt == 7),
                                         tile_position=(0, 32 * g),
                                         skip_group_check=True)

            flush_norm()
            pending = []
            for jt in range(8):
                k_sl = slice(bl * N + jt * 128, bl * N + (jt + 1) * 128)
                pss = [pspool.tile([128, 1024], DT.float32, name=f"ps{h0 + hi}{bl}{jt}", tag="ps")
                       for hi in range(2)]
                # scores first (adjacent different-band matmuls overlap in the
                # PE and their LDWEIGHTS pull ahead), then the full-array
                # identity matmuls that accumulate adj on top
                for hi, h in enumerate((h0, h0 + 1)):
                    g, hg = h % 4, h // 4
                    for ih in range(2):
                        i_sl = slice(ih * 512, (ih + 1) * 512)
                        q_sl = slice(bl * N + ih * 512, bl * N + (ih + 1) * 512)
                        nc.tensor.matmul(pss[hi][:, i_sl],
                                         k_band[32 * g:32 * g + 16, hg, k_sl],
                                         q_band[32 * g:32 * g + 16, hg, q_sl],
                                         start=True, stop=True,
                                         tile_position=(32 * g, 0))
                if len(pending) > 2:
                    emit_av(*pending.pop(0))
                at_mms = []
                for hi, h in enumerate((h0, h0 + 1)):
                    if jt < NJA:
                        ate = attnpool.tile([128, 1024], DT.bfloat16, name=f"ate{h}{bl}{jt}", tag="ate", bufs=3)
                        nc.scalar.activation(ate[:, :], pss[hi][:, :], AF.Exp)
                        at = attnpool.tile([128, 1024], DT.bfloat16, name=f"at{h}{bl}{jt}", tag="at")
                        nc.vector.scalar_tensor_tensor(at[:, :], ate[:, :], 1.0,
                                                       adjbs[hi][:, jt, :],
                                                       ALU.bypass, ALU.mult)
                        at_mms.append(at)
                    else:
                        ati = attnpool.tile([128, 1024], DT.int16, name=f"ati{h}{bl}{jt}", tag="ati")
                        nc.vector.scalar_tensor_tensor(ati[:, :], pss[hi][:, :], C1,
                                                       adjps[hi][:, jt - NJA, :],
                                                       ALU.mult, ALU.add)
                        at_mms.append(ati.bitcast(DT.bfloat16))
                pending.append((jt, at_mms))
            for pe_ in pending:
                emit_av(*pe_)
            # normalization: rec = 1/denominator via exp(-ln(|x|+eps)) on
            # ScalarE, then one k=128 selector matmul broadcasts each band's
            # denominator row to the whole band. ACT ops grouped by table set.
            stashes = []
            for ih in range(2):
                stash = smallpool.tile([128, 512], DT.bfloat16, name=f"stash{ih}", tag=f"stash{ih}", bufs=4)
                nc.scalar.copy(stash[:, :], pavs[ih][:, :])
                stashes.append(stash)
            absbs, lnbs, recs = [], [], []
            for ih in range(2):
                absb = smallpool.tile([128, 512], DT.float32, name=f"absb{ih}", tag=f"absb{ih}", bufs=2)
                nc.scalar.activation(absb[:, :], stashes[ih][:, :], AF.Abs)
                absbs.append(absb)
            for ih in range(2):
                lnb = smallpool.tile([128, 512], DT.float32, name=f"lnb{ih}", tag=f"lnb{ih}", bufs=2)
                nc.scalar.activation(lnb[:, :], absbs[ih][:, :], AF.Ln, bias=eps_c[:, :])
                lnbs.append(lnb)
            for ih in range(2):
                rec = smallpool.tile([128, 512], DT.bfloat16, name=f"rec{ih}", tag=f"rec{ih}", bufs=4)
                nc.scalar.activation(rec[:, :], lnbs[ih][:, :], AF.Exp, scale=-1.0)
                recs.append(rec)
            deferred_norms.append((h0, bl, recs, stashes))

    while deferred_norms:
        flush_norm(depth=0)

    # ---- output projection --------------------------------------------------
    out_sb = cpool.tile([128, NI], DT.float32)
    for t in range(NI // 512):
        sl = slice(t * 512, (t + 1) * 512)
        po = ppool2.tile([128, 512], DT.float32, name=f"po{t}", tag="p2")
        nc.tensor.matmul(po[:, :], woT_bf[:, :], avT[:, sl], start=True, stop=True)
        nc.scalar.activation(out_sb[:, sl], po[:, :], AF.Identity, bias=bo_t[:, :])
    nc.sync.dma_start(out=d["outT"][:, :], in_=out_sb[:, :])


def _patch_act_tables():
    """Force Exp/Ln/Identity to resolve to the one table set containing all of
    them, so interleaved exp (softmax) and ln/exp (reciprocal) don't thrash
    ACT_TABLE_LOADs (~2.7us each)."""
    if getattr(bacc, "_act_tables_patched", False):
        return
    orig = bacc.get_activation_tables

    def patched(arch):
        tabs = {k: set(v) for k, v in orig(arch).items()}
        want = {AF.Exp, AF.Ln, AF.Identity, AF.Square}
        combined = None
        for name, funcs in tabs.items():
            if want <= funcs:
                combined = name
                break
        if combined is not None:
            for name, funcs in tabs.items():
                if name != combined:
                    funcs -= want
        return tabs

    bacc.get_activation_tables = patched
    bacc._act_tables_patched = True


def _patch_ldw_opt():
    if getattr(bass_utils, "_ldw_patched", False):
        return
    orig = bass_utils.run_command

    def patched(argv, **kw):
        argv = ["--enable-ldw-opt=true" if a == "--enable-ldw-opt=false" else a
                for a in argv]
        return orig(argv, **kw)

    bass_utils.run_command = patched
    bass_utils._ldw_patched = True


def _build():
    if os.environ.get("KERNEL_LDW_OPT", "0") == "1":
        _patch_ldw_opt()
    nc = bacc.Bacc("TRN2", target_bir_lowering=False, debug=False, num_devices=N_CORES)
    io = {}
    io["xT"] = nc.dram_tensor("xT", [128, NI], DT.float32, kind="ExternalInput").ap()
    io["adjT"] = nc.dram_tensor("adjT", [HEADS, N, N], DT.float32, kind="ExternalInput").ap()
    io["wqk"] = nc.dram_tensor("wqk", [128, 256], DT.float32, kind="ExternalInput").ap()
    io["wvT"] = nc.dram_tensor("wvT", [128, 128], DT.float32, kind="ExternalInput").ap()
    io["woT"] = nc.dram_tensor("woT", [128, 128], DT.float32, kind="ExternalInput").ap()
    io["bq2"] = nc.dram_tensor("bq2", [128, 1], DT.float32, kind="ExternalInput").ap()
    io["bk2"] = nc.dram_tensor("bk2", [128, 1], DT.float32, kind="ExternalInput").ap()
    io["bo2"] = nc.dram_tensor("bo2", [128, 1], DT.float32, kind="ExternalInput").ap()
    io["bvr"] = nc.dram_tensor("bvr", [1, 128], DT.float32, kind="ExternalInput").ap()
    io["esel"] = nc.dram_tensor("esel", [128, 128], DT.float32, kind="ExternalInput").ap()
    io["outT"] = nc.dram_tensor("outT", [128, NI], DT.float32, kind="ExternalOutput").ap()
    nc._io_aps = io
    import contextlib
    with tile.TileContext(nc) as tc:
        with contextlib.ExitStack() as stack:
            _emit(nc, tc, stack)
    nc.compile()
    return nc


def kernel(x, Wq, bq, Wk, bk, Wv, bv, Wo, bo, adj):
    global _CACHED_NC, LAST_EXEC_NS
    x = np.asarray(x, np.float32)
    Wq, Wk, Wv, Wo = (np.asarray(w, np.float32) for w in (Wq, Wk, Wv, Wo))
    bq, bk, bv, bo = (np.asarray(b_, np.float32) for b_ in (bq, bk, bv, bo))
    adj = np.asarray(adj, np.float32)

    if _CACHED_NC is None:
        _CACHED_NC = _build()
    nc = _CACHED_NC

    adjT = np.ascontiguousarray(adj[0].transpose(0, 2, 1))      # (8, N, N), [h, j, i]
    wqk_np = np.ascontiguousarray(np.concatenate([Wq.T, Wk.T], axis=1))  # (128, 256)
    wvT_np = np.ascontiguousarray(Wv.T)
    woT_np = np.ascontiguousarray(Wo.T)
    shared = {
        "adjT": adjT,
        "wqk": wqk_np,
        "wvT": wvT_np,
        "woT": woT_np,
        "bq2": np.ascontiguousarray(bq.reshape(128, 1)),
        "bk2": np.ascontiguousarray(bk.reshape(128, 1)),
        "bo2": np.ascontiguousarray(bo.reshape(128, 1)),
        "bvr": np.ascontiguousarray(bv.reshape(1, 128)),
        "esel": np.ascontiguousarray(
            (np.arange(128)[:, None] == (32 * (np.arange(128)[None, :] // 32) + 16)
             ).astype(np.float32)),
    }
    in_maps = []
    for c in range(N_CORES):
        xT_c = np.ascontiguousarray(x[BL * c:BL * (c + 1)].reshape(NI, H).T)
        m = dict(shared)
        m["xT"] = xT_c
        in_maps.append(m)

    trace = os.environ.get("KERNEL_TRACE", "0") == "1"
    if trace:
        _install_ntff_hook()
    res = bass_utils.run_bass_kernel_spmd(nc, in_maps, core_ids=list(range(N_CORES)), trace=trace)
    LAST_EXEC_NS = res.exec_time_ns

    out = np.empty((B, N, H), np.float32)
    for c in range(N_CORES):
        oT = res.results[c]["outT"]
        for bl in range(BL):
            out[BL * c + bl] = oT[:, bl * N:(bl + 1) * N].T
    # softmax rows sum to 1 exactly => mean(|attn|) == 1/N (attn > 0)
    loss = np.float32(1e-4 / N)
    return out, loss
